# revision 45
# baseline (speedup 1.0000x reference)
"""BPCA pooling layer on 8 Trainium2 NeuronCores (Bass/Tile).

Math: per sample, the reference's `data = patches.reshape(-1, 4)` groups 4
consecutive channels (C=256 is divisible by 4), so `data` is exactly the
sample's contiguous buffer viewed as [N, 4] with N = H*W*C/4.  The layer is:

  1. per-column mean/std over N rows, dn = (data-mean)/std
  2. gram = dn^T dn (4x4), comp = top eigenvector (jnp.linalg.eigh)
  3. out = (dn @ comp) reshaped to [H/2, W/2, C] with channel permutation
     c' = (2*di+dj)*64 + (c//4)

Device plan (2 samples per core, pure data parallel).  Both passes are
DMA-bound, so all device I/O except the tiny stats tensor is fp16 --
quantizing x to fp16 perturbs the final output by ~3e-4 rel (measured
against the fixed seed), 60x under the 2e-2 gate, and halves traffic:

  pass 1: PE computes the 128x128 half-channel second-moment matrix
          M[j,j'] = sum_{pix,G} x[pix,128G+j]*x[pix,128G+j'] plus column
          sums (ones column), accumulated in fp32 PSUM, from fp16 inputs.
          128-col blocks (one matmul per 128-row block, N=130 moving)
          instead of 256-col halves: same LDWEIGHTS count but half the
          moving columns, so the PE stream (~81ns/MM warm) stays under
          the DMA stream.
  host:   fold M into the 4x4 gram (S_kl = sum_g M[4g+k,4g+l]), compute
          mean/std/gram in f64, eigh on CPU jax (same implementation the
          reference uses), derive w_k = comp_k/std_k and
          bias = -sum_k mean_k*comp_k/std_k.
  pass 2: out = sum_k x_k*w_k + bias as 4 tensor_scalar (4x DVE mode for
          fp16 step-1) + 3 tensor_tensor adds (2x mode) -- NOT the
          scalar_tensor_tensor chain, which has no accelerated DVE uops
          and runs 1 elem/cyc.  Host-built k-planes keep every access
          and DMA fully contiguous; stores issue from the ACT queue.
          The output channel permutation is folded into the host layout.

Loads alternate between the SP and Pool (gpsimd) DMA queues so the 16
DMA engines stay fed between descriptors.
"""

import numpy as np

# ---------------------------------------------------------------------------
# Problem constants (hardcoded per spec)
# ---------------------------------------------------------------------------
B, H, W, C = 16, 112, 112, 256
N_CORES = 8
SPC = B // N_CORES          # samples per core = 2
PIX = H * W                 # 12544 pixels per sample
NBLK1 = PIX * C // (128 * 128)  # 196 row-blocks of 128 per sample
BST1 = 130                  # per-block SBUF cols: 128 data + 1 ones + 1 pad
P1_TILES = [2, 4, 6, 8, 12] + [14] * 11 + [10]  # graduated DMA tiles
# (sum=196); small head so PE starts early, fine-grained so the whole-tile
# DMA-completion semaphore never stalls the PE for more than ~1.5us
NROWS = PIX * C // 4        # 802816 rows of the [N, 4] data matrix
HO, WO = H // 2, W // 2     # 56 x 56 output

_programs = None
_fused_program = None
LAST_PROFILE = {}
TRACE = False
TRACE_DIRS = {}
FUSED = True                # single-NEFF path (on-device eigensolve)
NSQ = 9                     # matrix squarings (power 512; sim err 8e-5)
P2_TILES = [1, 2, 4, 6, 6, 6, 6, 6, 6, 6]
CONST_COLS = 312


# ---------------------------------------------------------------------------
# TileContext with a walrus-compatible tail drain
# ---------------------------------------------------------------------------
def _make_tile_context(nc):
    from concourse.tile import TileContext
    return TileContext(nc)


def _split_sync_waits(nc):
    """walrus (CoreV2/V3 codegen) rejects instructions carrying more than 2
    sync commands (waits + updates combined); Tile freely emits e.g. 2 waits
    + 1 update.  Hoist excess waits onto same-engine NOPs inserted directly
    before the offending instruction -- same engine means the same program-
    order point, so semantics are unchanged."""
    import concourse.mybir as mybir

    def mint_nop(engine):
        inner = nc.engines[engine].nop().ins
        for blk in nc.m.functions[0].blocks:
            il = blk.instructions
            for k in range(len(il) - 1, -1, -1):
                if il[k] is inner:
                    il.pop(k)
                    return inner
        raise RuntimeError("minted nop not found in any block")

    for fn in nc.m.functions:
        for blk in fn.blocks:
            il = blk.instructions
            i = 0
            while i < len(il):
                inst = il[i]
                si = inst.sync_info
                waits = list(si.on_wait) if si and si.on_wait else []
                upds = list(si.on_update) if si and si.on_update else []
                # observed walrus limits: at most 1 wait per instruction
                # (1 wait + 1 update compiles; 2 waits anywhere does not)
                if len(waits) > 1:
                    extra, keep = waits[:-1], waits[-1:]
                    for wchunk in extra:
                        nop = mint_nop(inst.engine)
                        nop.sync_info = mybir.SyncInfo(
                            on_wait=[wchunk], on_update=[])
                        il.insert(i, nop)
                        i += 1
                    inst.sync_info = mybir.SyncInfo(
                        on_wait=keep, on_update=upds)
                i += 1


def _build_pass1():
    import concourse.bass as bass
    import concourse.mybir as mybir

    f32 = mybir.dt.float32
    f16 = mybir.dt.float16

    nc = bass.Bass("TRN2", target_bir_lowering=False, debug=False,
                   num_devices=N_CORES)
    # The host pre-interleaves a ones column per block (col 128 of each
    # 130-wide block) so one DMA loads data + ones and no on-device memset
    # is needed.
    x = nc.dram_tensor("x", [SPC, 128, NBLK1 * BST1], f16,
                       kind="ExternalInput").ap()
    stats = nc.dram_tensor("stats", [SPC, 128, BST1], f32,
                           kind="ExternalOutput").ap()

    with _make_tile_context(nc) as tc:
        with (
            tc.tile_pool(name="inp", bufs=4) as inp,
            tc.tile_pool(name="psum", bufs=2, space="PSUM") as psum,
            tc.tile_pool(name="sout", bufs=2) as soutp,
        ):
            qi = 0
            for s in range(SPC):
                ps = psum.tile([128, BST1], f32, tag="ps")
                # graduated tile sizes: tiny first tile so the PE starts
                # early instead of waiting on a large load
                b0 = 0
                for nb in P1_TILES:
                    t = inp.tile([128, nb * BST1], f16, tag="in")
                    t3 = t[:].rearrange("p (j b) -> p j b", b=BST1)
                    q = (nc.sync, nc.gpsimd, nc.scalar)[qi % 3]
                    qi += 1
                    q.dma_start(
                        out=t[:],
                        in_=x[s, :, b0 * BST1:(b0 + nb) * BST1])
                    for j in range(nb):
                        first = b0 + j == 0
                        last = b0 + j == NBLK1 - 1
                        nc.tensor.matmul(ps[:, 0:BST1],
                                         t3[:, j:j + 1, 0:128],
                                         t3[:, j:j + 1, 0:BST1],
                                         start=first, stop=last,
                                         skip_group_check=True)
                    b0 += nb
                so = soutp.tile([128, BST1], f32)
                nc.vector.tensor_copy(out=so[:], in_=ps[:, 0:BST1])
                # ACT-issued DMA: keeps the load queues free to prefetch
                nc.scalar.dma_start(out=stats[s], in_=so[:])
    _split_sync_waits(nc)
    return nc


def _build_pass2():
    import concourse.bass as bass
    import concourse.mybir as mybir

    f16 = mybir.dt.float16
    alu = mybir.AluOpType
    act_copy = mybir.ActivationFunctionType.Copy
    OO = 49  # output pixels per partition (3136 = 64 partitions x 49)

    nc = bass.Bass("TRN2", target_bir_lowering=False, debug=False,
                   num_devices=N_CORES)
    # Partition p = (s_local*64 + p64): both samples fill 128 partitions so
    # every DMA is a fully-contiguous 128-partition transfer (~420 GB/s).
    # Host pre-transposes to per-tile k-plane blocks:
    #   x[:, off_t + (k*oo_t + oo)*C + c']
    f32 = mybir.dt.float32
    x = nc.dram_tensor("x", [128, OO * 4 * C], f16,
                       kind="ExternalInput").ap()
    wb = nc.dram_tensor("wb", [128, 8], f32, kind="ExternalInput").ap()
    out = nc.dram_tensor("out", [128, OO * C], f16,
                         kind="ExternalOutput").ap()

    with _make_tile_context(nc) as tc:
        with (
            tc.tile_pool(name="w", bufs=1) as wpool,
            tc.tile_pool(name="inp", bufs=3) as inp,
            tc.tile_pool(name="acc", bufs=2) as accp,
        ):
            wt = wpool.tile([128, 8], f32, tag="wb")
            nc.sync.dma_start(out=wt[:], in_=wb[:])
            # touch wb on DVE so later DVE ops inherit the dep by program
            # order instead of each carrying a sem wait
            wl = wpool.tile([128, 8], f32, tag="wl")
            nc.vector.tensor_copy(out=wl[:], in_=wt[:])
            w = [wl[:, k:k + 1] for k in range(4)]
            bias = wl[:, 4:5]
            off = 0
            ooff = 0
            qi = 0
            # tiny first tile: DVE starts ~10us earlier
            for oo in [1, 8, 8, 8, 8, 8, 8]:
                F = oo * C
                it = inp.tile([128, 4 * F], f16, tag="it")
                q = (nc.sync, nc.gpsimd, nc.scalar)[qi % 3]
                qi += 1
                q.dma_start(out=it[:], in_=x[:, off:off + 4 * F])
                t0 = accp.tile([128, F], f16, tag="t0")
                t1 = accp.tile([128, F], f16, tag="t1")
                t2 = accp.tile([128, F], f16, tag="t2")
                t3 = accp.tile([128, F], f16, tag="t3")
                u0 = accp.tile([128, F], f16, tag="u0")
                u1 = accp.tile([128, F], f16, tag="u1")
                ot = accp.tile([128, F], f16, tag="ot")
                # t_k = x_k*w_k (tensor_scalar: 4x DVE mode for fp16
                # step-1), bias folded into t0; pairwise tensor_tensor
                # adds (2x mode).  ACT offload measured SLOWER (fp16
                # ACTIVATE runs 1 elem/cyc, 2.2us/op) -- keep all on DVE.
                nc.vector.tensor_scalar(
                    t0[:], it[:, 0:F], w[0], bias,
                    op0=alu.mult, op1=alu.add)
                nc.vector.tensor_scalar(
                    t1[:], it[:, F:2 * F], w[1], None, op0=alu.mult)
                nc.vector.tensor_tensor(
                    u0[:], t0[:], t1[:], op=alu.add)
                nc.vector.tensor_scalar(
                    t2[:], it[:, 2 * F:3 * F], w[2], None, op0=alu.mult)
                nc.vector.tensor_scalar(
                    t3[:], it[:, 3 * F:4 * F], w[3], None, op0=alu.mult)
                nc.vector.tensor_tensor(
                    u1[:], t2[:], t3[:], op=alu.add)
                nc.vector.tensor_tensor(
                    ot[:], u0[:], u1[:], op=alu.add)
                # ACT-issued store (only SP/ACT/gpsimd can start DMAs;
                # the trigger costs ~0.6us, ACT has headroom under DVE)
                nc.scalar.dma_start(
                    out=out[:, ooff:ooff + F], in_=ot[:])
                off += 4 * F
                ooff += F
    _split_sync_waits(nc)
    return nc


def _host_consts():
    """Constant tensor for the fused kernel's on-device fold/eigensolve."""
    ct = np.zeros((128, CONST_COLS), np.float32)
    p = np.arange(128)
    # 0:130 block-diag mask for M -> per-group fold (col 128 = chansums)
    q = np.arange(128)
    ct[:, 0:128] = (p[:, None] // 4 == q[None, :] // 4).astype(np.float32)
    ct[:, 128] = 1.0
    # 130:138 / 138:146: per-sample fold selectors (PSUM-accumulated)
    for m in range(4):
        ct[:, 130 + m] = (p % 4 == m)
        ct[:, 138 + 4 + m] = 0.0  # placeholder; filled below
    for m in range(4):
        ct[:, 138 + m] = 0.0
        ct[:, 142 + m] = (p % 4 == m)
    # 146:154 I8
    ct[0:8, 146:154] = np.eye(8, dtype=np.float32)
    # 154:162 blockones8
    p8 = np.arange(8)
    ct[0:8, 154:162] = (p8[:, None] // 4 == p8[None, :] // 4)
    # 162 r8 (fixed probe vector, repeated per sample)
    r = np.array([0.5393, -0.2117, 0.8313, 0.1078], np.float32)
    ct[0:8, 162] = r[p8 % 4]
    # 163 selA (sample 0 partitions), 164 selB (sample 1)
    ct[:, 163] = (p < 64)
    ct[:, 164] = (p >= 64)
    # 165:293 ones-row on partition 0 (K=1 broadcast matmul lhsT; matmul
    # requires lhsT base partition in {0, 32, 64})
    ct[0, 165:293] = 1.0
    # 294:302 / 302:310: half-diagonal projectors for the block-diag embed
    ct[0:8, 294:302] = np.diag((np.arange(8) < 4).astype(np.float32))
    ct[0:8, 302:310] = np.diag((np.arange(8) >= 4).astype(np.float32))
    return ct


def _build_fused():
    import concourse.bass as bass
    import concourse.mybir as mybir

    f32 = mybir.dt.float32
    f16 = mybir.dt.float16
    alu = mybir.AluOpType
    actf = mybir.ActivationFunctionType
    ax = mybir.AxisListType
    OO = 49

    nc = bass.Bass("TRN2", target_bir_lowering=False, debug=False,
                   num_devices=N_CORES)
    xg = nc.dram_tensor("xg", [SPC, 128, NBLK1 * BST1], f16,
                        kind="ExternalInput").ap()
    xp = nc.dram_tensor("xp", [128, OO * 4 * C], f16,
                        kind="ExternalInput").ap()
    cst = nc.dram_tensor("cst", [128, CONST_COLS], f32,
                         kind="ExternalInput").ap()
    out = nc.dram_tensor("out", [128, OO * C], f16,
                         kind="ExternalOutput").ap()
    stats = nc.dram_tensor("stats", [SPC, 128, BST1], f32,
                           kind="ExternalOutput").ap()
    wdev = nc.dram_tensor("wdev", [128, 8], f32,
                          kind="ExternalOutput").ap()
    gate = nc.dram_tensor("gate", [128, 4], f32,
                          kind="ExternalOutput").ap()

    with _make_tile_context(nc) as tc:
        with (
            tc.tile_pool(name="cstp", bufs=1) as cstp,
            tc.tile_pool(name="inp", bufs=8) as inp,
            tc.tile_pool(name="pin", bufs=1) as pin,
            tc.tile_pool(name="psum", bufs=1, space="PSUM") as psum,
            tc.tile_pool(name="pst", bufs=1, space="PSUM") as pst,
            tc.tile_pool(name="sml", bufs=1) as sml,
            tc.tile_pool(name="eig", bufs=3) as eig,
            tc.tile_pool(name="accA", bufs=1) as accp,
            tc.tile_pool(name="accB", bufs=2) as accb,
        ):
            ct = cstp.tile([128, CONST_COLS], f32, tag="cst")
            nc.sync.dma_start(out=ct[:], in_=cst[:])
            mask130 = ct[:, 0:130]
            lsel0 = ct[:, 130:138]
            lsel1 = ct[:, 138:146]
            lseld = ct[0:8, 130:134]     # (p%4==l) diag selector rows 0-7
            i8 = ct[0:8, 146:154]
            m8 = ct[0:8, 154:162]
            r8 = ct[0:8, 162:163]
            selA = ct[:, 163:164]
            selB = ct[:, 164:165]
            onesrow = ct[0:1, 165:293]

            # bulk loads only on the SP/gpsimd queues: the ACT queue gets
            # visibly worse DMA-engine service (~90GB/s) and a lagging
            # tile stalls the in-order PE stream
            QS = (nc.sync, nc.gpsimd)
            qi = 0
            # ---- phase 1: gram matmuls over the site-block stream -------
            pss = []
            for s in range(SPC):
                ps = psum.tile([128, BST1], f32, tag=f"ps{s}")
                b0 = 0
                for nb in P1_TILES:
                    t = inp.tile([128, nb * BST1], f16, tag="in")
                    t3 = t[:].rearrange("p (j b) -> p j b", b=BST1)
                    QS[qi % 2].dma_start(
                        out=t[:], in_=xg[s, :, b0 * BST1:(b0 + nb) * BST1])
                    qi += 1
                    for j in range(nb):
                        nc.tensor.matmul(ps[:, 0:BST1],
                                         t3[:, j:j + 1, 0:128],
                                         t3[:, j:j + 1, 0:BST1],
                                         start=(b0 + j == 0),
                                         stop=(b0 + j == NBLK1 - 1),
                                         skip_group_check=True)
                    b0 += nb
                pss.append(ps)

            # ---- fold part 1: PSUM -> SBUF copies ----------------------
            sos = []
            for s in range(SPC):
                so = sml.tile([128, BST1], f32, tag=f"so{s}")
                nc.vector.tensor_copy(out=so[:], in_=pss[s][:, 0:BST1])
                sos.append(so)

            # ---- gate stores: one per DMA queue, each waiting on the
            # gram result, so the plane-load triggers queued behind them
            # cannot compete with the gram stream for DMA engines --------
            nc.sync.dma_start(out=stats[0], in_=sos[0][:])
            nc.gpsimd.dma_start(out=stats[1], in_=sos[1][:])
            nc.scalar.dma_start(out=gate[:], in_=sos[1][:, 0:4])

            # ---- queue all plane loads (strictly after the gram loads) --
            planes = []
            off = 0
            for oo in P2_TILES:
                F = oo * C
                it = pin.tile([128, 4 * F], f16, tag=f"it{len(planes)}")
                QS[qi % 2].dma_start(out=it[:], in_=xp[:, off:off + 4 * F])
                qi += 1
                planes.append((it, F))
                off += 4 * F

            # ---- fold part 2: batched [8,*] moments --------------------
            t1p = pst.tile([8, BST1], f32, tag="t1p")
            for s in range(SPC):
                bm = sml.tile([128, BST1], f32, tag=f"bm{s}")
                nc.vector.tensor_tensor(bm[:], sos[s][:], mask130,
                                        op=alu.mult)
                nc.tensor.matmul(t1p[:], lsel0 if s == 0 else lsel1,
                                 bm[:], start=(s == 0), stop=(s == SPC - 1),
                                 skip_group_check=True)
            t1b = sml.tile([8, BST1], f32, tag="t1b")
            nc.vector.tensor_copy(out=t1b[:], in_=t1p[:])
            s8 = sml.tile([8, 4], f32, tag="s8")
            nc.vector.tensor_reduce(
                s8[:], t1b[:, 0:128].rearrange("p (g l) -> p l g", l=4),
                axis=ax.X, op=alu.add)
            mu8 = sml.tile([8, 1], f32, tag="mu8")
            nc.vector.tensor_scalar(mu8[:], t1b[:, 128:129],
                                    float(1.0 / NROWS), None, op0=alu.mult)
            # block-diag embed of the two 4x4 S matrices via masked PE
            # matmuls (DVE cannot address partition ranges off base 0)
            sembp = pst.tile([8, 8], f32, tag="o8")
            nc.tensor.matmul(sembp[:, 0:4], ct[0:8, 294:302], s8[:],
                             start=True, stop=True)
            nc.tensor.matmul(sembp[:, 4:8], ct[0:8, 302:310], s8[:],
                             start=True, stop=True)
            semb = sml.tile([8, 8], f32, tag="semb")
            nc.vector.tensor_copy(out=semb[:], in_=sembp[:])
            tmp84 = sml.tile([8, 4], f32, tag="tmp84")
            nc.vector.tensor_tensor(tmp84[:], s8[:], lseld, op=alu.mult)
            e2 = sml.tile([8, 1], f32, tag="e2")
            nc.vector.tensor_reduce(e2[:], tmp84[:], axis=ax.X, op=alu.add)
            e2n = sml.tile([8, 1], f32, tag="e2n")
            nc.vector.tensor_scalar(e2n[:], e2[:], float(1.0 / NROWS), None,
                                    op0=alu.mult)
            m2 = sml.tile([8, 1], f32, tag="m2")
            nc.vector.tensor_tensor(m2[:], mu8[:], mu8[:], op=alu.mult)
            var8 = sml.tile([8, 1], f32, tag="var8")
            nc.vector.tensor_tensor(var8[:], e2n[:], m2[:], op=alu.subtract)
            std8 = sml.tile([8, 1], f32, tag="std8")
            nc.scalar.activation(std8[:], var8[:], actf.Sqrt)
            # outer products via PE transpose + K=1 matmul
            mutp = pst.tile([1, 8], f32, tag="tr")
            nc.tensor.transpose(mutp[:], mu8[:], i8)
            mut = sml.tile([1, 8], f32, tag="mut")
            nc.vector.tensor_copy(out=mut[:], in_=mutp[:])
            stdtp = pst.tile([1, 8], f32, tag="tr")
            nc.tensor.transpose(stdtp[:], std8[:], i8)
            stdt = sml.tile([1, 8], f32, tag="stdt")
            nc.vector.tensor_copy(out=stdt[:], in_=stdtp[:])
            omup = pst.tile([8, 8], f32, tag="o8")
            nc.tensor.matmul(omup[:], mut[:], mut[:], start=True, stop=True)
            omu = sml.tile([8, 8], f32, tag="omu")
            nc.vector.tensor_copy(out=omu[:], in_=omup[:])
            ostdp = pst.tile([8, 8], f32, tag="o8")
            nc.tensor.matmul(ostdp[:], stdt[:], stdt[:], start=True,
                             stop=True)
            ostd = sml.tile([8, 8], f32, tag="ostd")
            nc.vector.tensor_copy(out=ostd[:], in_=ostdp[:])
            gnum = sml.tile([8, 8], f32, tag="gnum")
            nc.vector.scalar_tensor_tensor(gnum[:], omu[:],
                                           float(-NROWS), semb[:],
                                           op0=alu.mult, op1=alu.add)
            rostd = sml.tile([8, 8], f32, tag="rostd")
            nc.vector.reciprocal(rostd[:], ostd[:])
            g8 = sml.tile([8, 8], f32, tag="g8")
            nc.vector.tensor_tensor(g8[:], gnum[:], rostd[:], op=alu.mult)
            g8m = sml.tile([8, 8], f32, tag="g8m")
            nc.vector.tensor_tensor(g8m[:], g8[:], m8, op=alu.mult)
            a8 = eig.tile([8, 8], f32, tag="a8")
            nc.vector.scalar_tensor_tensor(a8[:], i8,
                                           float(-0.98 * NROWS), g8m[:],
                                           op0=alu.mult, op1=alu.add)

            # ---- eigensolve: repeated squaring with per-block fro norm --
            for t in range(NSQ):
                if t in (0, 5):
                    sq = eig.tile([8, 8], f32, tag="sq")
                    nc.vector.tensor_tensor(sq[:], a8[:], a8[:],
                                            op=alu.mult)
                    rs = eig.tile([8, 1], f32, tag="rs")
                    nc.vector.tensor_reduce(rs[:], sq[:], axis=ax.X,
                                            op=alu.add)
                    fbp = pst.tile([8, 1], f32, tag="v1")
                    nc.tensor.matmul(fbp[:], m8, rs[:], start=True,
                                     stop=True)
                    fb = eig.tile([8, 1], f32, tag="fb")
                    nc.vector.tensor_copy(out=fb[:], in_=fbp[:])
                    frt = eig.tile([8, 1], f32, tag="frt")
                    nc.scalar.activation(frt[:], fb[:], actf.Sqrt)
                    rfrt = eig.tile([8, 1], f32, tag="rfrt")
                    nc.vector.reciprocal(rfrt[:], frt[:])
                    an = eig.tile([8, 8], f32, tag="a8")
                    nc.vector.tensor_scalar(an[:], a8[:], rfrt[:], None,
                                            op0=alu.mult)
                    a8 = an
                a2p = pst.tile([8, 8], f32, tag="o8")
                nc.tensor.matmul(a2p[:], a8[:], a8[:], start=True,
                                 stop=True)
                a8 = eig.tile([8, 8], f32, tag="a8")
                nc.vector.tensor_copy(out=a8[:], in_=a2p[:])

            # ---- top eigenvector, w, bias ------------------------------
            v8p = pst.tile([8, 1], f32, tag="v1")
            nc.tensor.matmul(v8p[:], a8[:], r8, start=True, stop=True)
            v8 = sml.tile([8, 1], f32, tag="v8")
            nc.vector.tensor_copy(out=v8[:], in_=v8p[:])
            vsq = sml.tile([8, 1], f32, tag="vsq")
            nc.vector.tensor_tensor(vsq[:], v8[:], v8[:], op=alu.mult)
            nbp = pst.tile([8, 1], f32, tag="v1")
            nc.tensor.matmul(nbp[:], m8, vsq[:], start=True, stop=True)
            nb8 = sml.tile([8, 1], f32, tag="nb8")
            nc.vector.tensor_copy(out=nb8[:], in_=nbp[:])
            nrt = sml.tile([8, 1], f32, tag="nrt")
            nc.scalar.activation(nrt[:], nb8[:], actf.Sqrt)
            rnrt = sml.tile([8, 1], f32, tag="rnrt")
            nc.vector.reciprocal(rnrt[:], nrt[:])
            vn8 = sml.tile([8, 1], f32, tag="vn8")
            nc.vector.tensor_scalar(vn8[:], v8[:], rnrt[:], None,
                                    op0=alu.mult)
            rstd8 = sml.tile([8, 1], f32, tag="rstd8")
            nc.vector.reciprocal(rstd8[:], std8[:])
            w8 = sml.tile([8, 1], f32, tag="w8")
            nc.vector.tensor_tensor(w8[:], vn8[:], rstd8[:], op=alu.mult)
            prod = sml.tile([8, 1], f32, tag="prod")
            nc.vector.tensor_tensor(prod[:], mu8[:], w8[:], op=alu.mult)
            pbp = pst.tile([8, 1], f32, tag="v1")
            nc.tensor.matmul(pbp[:], m8, prod[:], start=True, stop=True)
            pb = sml.tile([8, 1], f32, tag="pb")
            nc.vector.tensor_copy(out=pb[:], in_=pbp[:])
            bias8 = sml.tile([8, 1], f32, tag="bias8")
            nc.vector.tensor_scalar(bias8[:], pb[:], -1.0, None,
                                    op0=alu.mult)

            # ---- broadcast w/bias to the 128 projection partitions -----
            wtp = pst.tile([1, 8], f32, tag="tr")
            nc.tensor.transpose(wtp[:], w8[:], i8)
            btp = pst.tile([1, 8], f32, tag="tr")
            nc.tensor.transpose(btp[:], bias8[:], i8)
            wt16 = sml.tile([1, 16], f32, tag="wt16")
            nc.vector.tensor_copy(out=wt16[:, 0:8], in_=wtp[:])
            nc.vector.tensor_copy(out=wt16[:, 8:16], in_=btp[:])
            wbalp = pst.tile([128, 16], f32, tag="wbalp")
            nc.tensor.matmul(wbalp[:], onesrow, wt16[:],
                             start=True, stop=True)
            wball = sml.tile([128, 16], f32, tag="wball")
            nc.vector.tensor_copy(out=wball[:], in_=wbalp[:])
            wa = sml.tile([128, 4], f32, tag="wa")
            nc.vector.tensor_scalar(wa[:], wball[:, 0:4], selA, None,
                                    op0=alu.mult)
            wb_ = sml.tile([128, 4], f32, tag="wb_")
            nc.vector.tensor_scalar(wb_[:], wball[:, 4:8], selB, None,
                                    op0=alu.mult)
            wl4 = sml.tile([128, 4], f32, tag="wl4")
            nc.vector.tensor_tensor(wl4[:], wa[:], wb_[:], op=alu.add)
            ba = sml.tile([128, 1], f32, tag="ba")
            nc.vector.tensor_scalar(ba[:], wball[:, 8:9], selA, None,
                                    op0=alu.mult)
            bb2 = sml.tile([128, 1], f32, tag="bb2")
            nc.vector.tensor_scalar(bb2[:], wball[:, 12:13], selB, None,
                                    op0=alu.mult)
            bias128 = sml.tile([128, 1], f32, tag="bias128")
            nc.vector.tensor_tensor(bias128[:], ba[:], bb2[:], op=alu.add)

            # ---- wdev store (host compares sign against its own eigh) --
            nc.scalar.dma_start(out=wdev[:, 0:4], in_=wl4[:])
            nc.scalar.dma_start(out=wdev[:, 4:5], in_=bias128[:])

            # ---- projection (pass-2 compute) ---------------------------
            w = [wl4[:, k:k + 1] for k in range(4)]
            bias = bias128[:, 0:1]
            ooff = 0
            for it, F in planes:
                t0 = accp.tile([128, F], f16, tag="t0")
                t1 = accp.tile([128, F], f16, tag="t1")
                t2 = accp.tile([128, F], f16, tag="t2")
                t3 = accp.tile([128, F], f16, tag="t3")
                u0 = accp.tile([128, F], f16, tag="u0")
                u1 = accp.tile([128, F], f16, tag="u1")
                ot = accb.tile([128, F], f16, tag="ot")
                nc.vector.tensor_scalar(t0[:], it[:, 0:F], w[0], bias,
                                        op0=alu.mult, op1=alu.add)
                nc.vector.tensor_scalar(t1[:], it[:, F:2 * F], w[1], None,
                                        op0=alu.mult)
                nc.vector.tensor_tensor(u0[:], t0[:], t1[:], op=alu.add)
                nc.vector.tensor_scalar(t2[:], it[:, 2 * F:3 * F], w[2],
                                        None, op0=alu.mult)
                nc.vector.tensor_scalar(t3[:], it[:, 3 * F:4 * F], w[3],
                                        None, op0=alu.mult)
                nc.vector.tensor_tensor(u1[:], t2[:], t3[:], op=alu.add)
                nc.vector.tensor_tensor(ot[:], u0[:], u1[:], op=alu.add)
                QS[qi % 2].dma_start(out=out[:, ooff:ooff + F], in_=ot[:])
                qi += 1
                ooff += F
    _split_sync_waits(nc)
    return nc


def _get_programs():
    global _programs
    if _programs is None:
        _programs = (_build_pass1(), _build_pass2())
    return _programs


def _get_fused():
    global _fused_program
    if _fused_program is None:
        _fused_program = _build_fused()
    return _fused_program


def _host_middle(stats):
    """stats: [B, 128, 130] f32 -> w [B, 4] f64, bias [B] f64.

    Follows the reference downstream exactly: gram from (S - N mu mu^T) /
    (sigma sigma^T), comp = eigh(gram f32) top eigenvector on CPU jax.
    """
    stats = stats.astype(np.float64)
    M = stats[:, :, :128]                                # [B, 128, 128]
    chansum = stats[:, :, 128]                           # [B, 128]

    # fold channels j = 4g+k into columns k
    Mg = M.reshape(B, 32, 4, 32, 4)
    S = np.einsum("bgkgl->bkl", Mg)                      # [B, 4, 4]
    colsum = chansum.reshape(B, 32, 4).sum(axis=1)       # [B, 4]

    mu = colsum / NROWS
    e2 = np.einsum("bkk->bk", S) / NROWS
    var = np.maximum(e2 - mu * mu, 0.0)
    sigma = np.sqrt(var)
    denom = sigma[:, :, None] * sigma[:, None, :]
    gram = (S - NROWS * mu[:, :, None] * mu[:, None, :])
    with np.errstate(divide="ignore", invalid="ignore"):
        gram = np.where(denom > 0, gram / np.where(denom > 0, denom, 1.0), 0.0)

    # eigh with the same implementation/backend the reference uses (CPU jax)
    import jax
    import jax.numpy as jnp
    with jax.default_device(jax.devices("cpu")[0]):
        V = np.asarray(jnp.linalg.eigh(jnp.asarray(gram, jnp.float32))[1])
    comp = V[:, :, -1].astype(np.float64)                # top eigenvector

    with np.errstate(divide="ignore", invalid="ignore"):
        w = np.where(sigma > 0, comp / np.where(sigma > 0, sigma, 1.0), 0.0)
    bias = -(mu * w).sum(axis=1)
    return w, bias


def _layouts(x16):
    """Build the gram-pass and plane-pass device layouts from fp16 x."""
    xp = np.zeros((B, 128, NBLK1, BST1), np.float16)
    xp[..., :128] = x16.reshape(B, NBLK1, 128, 128).transpose(0, 2, 1, 3)
    xp[..., 128] = 1.0
    xp = xp.reshape(B, 128, NBLK1 * BST1)

    xpl = x16.reshape(B, HO, 2, WO, 2, C // 4, 4).transpose(
        0, 1, 3, 6, 2, 4, 5)
    xpl = np.ascontiguousarray(xpl).reshape(B, 64, 49, 4, C)
    segs = []
    oo0 = 0
    for oo in P2_TILES:
        seg = xpl[:, :, oo0:oo0 + oo].transpose(0, 1, 3, 2, 4)
        segs.append(seg.reshape(B, 64, 4 * oo * C))
        oo0 += oo
    x2h = np.concatenate(segs, axis=2)             # [B, 64, 49*4*C]
    return xp, x2h


def _kernel_fused(x16):
    from concourse.bass_utils import run_bass_kernel_spmd

    ncf = _get_fused()
    core_ids = list(range(N_CORES))
    xp, x2h = _layouts(x16)
    cst = _host_consts()
    ins = []
    for c in range(N_CORES):
        pair = x2h[c * SPC:(c + 1) * SPC]
        ins.append({
            "xg": xp[c * SPC:(c + 1) * SPC],
            "xp": pair.reshape(128, 49 * 4 * C),
            "cst": cst,
        })
    kw = dict(trace=True, tmpdir=TRACE_DIRS.get("pass1")) if TRACE else {}
    r = run_bass_kernel_spmd(ncf, ins, core_ids, **kw)
    if TRACE:
        LAST_PROFILE["pass1_ns"] = r.exec_time_ns
        LAST_PROFILE["pass2_ns"] = 0

    stats = np.concatenate([r.results[c]["stats"] for c in range(N_CORES)])
    w_host, _ = _host_middle(stats)

    outs = []
    for c in range(N_CORES):
        o = r.results[c]["out"].astype(np.float32).reshape(SPC, HO, WO, C)
        wdev = r.results[c]["wdev"]
        for s in range(SPC):
            wd = wdev[s * 64, 0:4].astype(np.float64)
            if np.dot(wd, w_host[c * SPC + s]) < 0:
                o[s] = -o[s]
        outs.append(o)
    return np.ascontiguousarray(np.concatenate(outs))


def kernel(x):
    from concourse.bass_utils import run_bass_kernel_spmd

    x = np.asarray(x)
    assert x.shape == (B, H, W, C), x.shape
    x16 = np.ascontiguousarray(x, dtype=np.float16)
    if FUSED:
        return _kernel_fused(x16)
    nc1, nc2 = _get_programs()
    core_ids = list(range(N_CORES))

    # pass-1 input: 128-row x 128-col blocks padded to 130 cols with a
    # ones column at 128, laid out exactly like the SBUF tiles
    # ([128 partitions, blocks]); row r = pix*2 + channel-half
    xp = np.zeros((B, 128, NBLK1, BST1), np.float16)
    xp[..., :128] = x16.reshape(B, NBLK1, 128, 128).transpose(0, 2, 1, 3)
    xp[..., 128] = 1.0
    xp = xp.reshape(B, 128, NBLK1 * BST1)
    in1 = [{"x": xp[c * SPC:(c + 1) * SPC]} for c in range(N_CORES)]
    kw1 = dict(trace=True, tmpdir=TRACE_DIRS.get("pass1")) if TRACE else {}
    r1 = run_bass_kernel_spmd(nc1, in1, core_ids, **kw1)
    if TRACE:
        LAST_PROFILE["pass1_ns"] = r1.exec_time_ns
    stats = np.concatenate([r1.results[c]["stats"] for c in range(N_CORES)])

    w, bias = _host_middle(stats)
    wbs = []
    for c in range(N_CORES):
        a = np.zeros((128, 8), np.float32)
        for s in range(SPC):
            b = c * SPC + s
            a[s * 64:(s + 1) * 64, 0:4] = w[b].astype(np.float32)
            a[s * 64:(s + 1) * 64, 4] = np.float32(bias[b])
        wbs.append(a)

    # pass-2 input: k-plane transpose, output-pixel-major.
    #   xplanes[s, outpix=(hi*56+wi), k, c'=(2di+dj)*64+j] = x[s,2hi+di,2wi+dj,4j+k]
    # outpix = p64*49 + oo; per tile t (oo block) the free layout is
    # [k, oo_t, c'], tiles concatenated along the free axis
    xpl = x16.reshape(B, HO, 2, WO, 2, C // 4, 4).transpose(0, 1, 3, 6, 2, 4, 5)
    xpl = np.ascontiguousarray(xpl).reshape(B, 64, 49, 4, C)
    segs = []
    oo0 = 0
    for oo in [1, 8, 8, 8, 8, 8, 8]:
        seg = xpl[:, :, oo0:oo0 + oo].transpose(0, 1, 3, 2, 4)
        segs.append(seg.reshape(B, 64, 4 * oo * C))
        oo0 += oo
    x2h = np.concatenate(segs, axis=2)             # [B, 64, 49*4*C]
    in2 = []
    for c in range(N_CORES):
        pair = x2h[c * SPC:(c + 1) * SPC]          # [2, 64, 49*4*C]
        in2.append({"x": pair.reshape(128, 49 * 4 * C), "wb": wbs[c]})
    kw2 = dict(trace=True, tmpdir=TRACE_DIRS.get("pass2")) if TRACE else {}
    r2 = run_bass_kernel_spmd(nc2, in2, core_ids, **kw2)
    if TRACE:
        LAST_PROFILE["pass2_ns"] = r2.exec_time_ns

    # gather: out[s*64+p64, oo*C+c'], outpix = p64*49+oo -> [B, HO, WO, C]
    outs = [r2.results[c]["out"].astype(np.float32).reshape(SPC, HO, WO, C)
            for c in range(N_CORES)]
    return np.ascontiguousarray(np.concatenate(outs))

